# revision 25
# baseline (speedup 1.0000x reference)
"""Cross-attention kernel for 8 TRN2 NeuronCores.

Reference computation (per batch b, c=1024 tokens, dim=1024):
    q = xf @ Wq.T ; k,v = cf @ Wkv.T split
    out = softmax(q @ k.T / 32) @ v

Key algebraic optimization (host-side weight folding):
    q @ k.T = xf @ (Wq.T @ Wk) @ cf.T  =  (xf @ M) @ cf.T
so the k-projection disappears entirely; M = Wq.T @ Wk is a fixed
weight matrix folded on the host (input-independent preprocessing).
Per batch the device does 4 (not 5) 1024^3 matmuls:
    aT[o,i] = M.T  @ xT           (lhsT=M[d,o],   rhs=xT[d,i])
    v[j,o]  = cT.T @ WvT          (lhsT=cT[d,j],  rhs=WvT[d,o])
    ST[j,i] = cT.T @ aT           (scores, transposed; c's raw feature
                                   axis IS the contraction axis post-fold)
    ET      = exp(ST/32)          (ACT, scale fused; logits ~N(0,1))
    out'[i,o] = ET.T @ v          (lhsT=ET[j,i], rhs=v[j,o])
    l[i]      = ET.T @ ones       (N=1 matmuls sharing the stationary tiles)
    out[i,o]  = out' * (1/l)      (DVE per-partition scale, fp16 out DMA)

Sharding: data-parallel over batch (16 batches -> 2 per core), SPMD on 8
cores, no collectives.  Activations enter pre-transposed (host-side) so
every matmul has its contraction dim on SBUF partitions.  The transposed
scores layout means the softmax matrix is never transposed on device.
"""

import os
import sys

import numpy as np


def _ensure_paths():
    for p in ("/opt/trn_rl_repo", "/root/.axon_site/_ro/trn_rl_repo"):
        if os.path.isdir(p) and p not in sys.path:
            sys.path.append(p)


try:
    import concourse.bass  # noqa: F401
except ImportError:
    _ensure_paths()

import concourse.bass as bass  # noqa: E402
import concourse.tile as tile  # noqa: E402
from concourse import bacc, mybir  # noqa: E402
from concourse import bass_utils  # noqa: E402

B, C, HH, WW = 16, 1024, 32, 32
D = HH * WW  # 1024
NCORES = 8
BPC = B // NCORES  # 2 batches per core
P = 128
KS = D // P  # 8 contraction subtiles
NT = C // P  # 8 row tiles
NH = 512  # matmul moving free dim (one PSUM bank)
SCALE = float(D) ** -0.5

CDT = mybir.dt.float16  # on-device compute dtype
NPDT = np.float16

F32 = mybir.dt.float32

WARMUP_MMS = int(os.environ.get("KERNEL_WARMUP_MMS", "20"))


def _emit(tc, xT, cT, m, wv, out):
    nc = tc.nc
    from contextlib import ExitStack

    ctx = ExitStack()
    with ctx:
        wpool = ctx.enter_context(tc.tile_pool(name="weights", bufs=1))
        # bufs=1: batch-1 x/c DMAs reuse batch-0 buffers, so Tile holds them
        # (WAR dependency) until batch-0's reads finish -- keeping the 4 MB
        # of batch-1 input traffic out of the startup-critical DMA window.
        iopool = ctx.enter_context(tc.tile_pool(name="io", bufs=1))
        actpool = ctx.enter_context(tc.tile_pool(name="acts", bufs=1))
        outpool = ctx.enter_context(tc.tile_pool(name="outs", bufs=3))
        smpool = ctx.enter_context(tc.tile_pool(name="small", bufs=2))
        psum = ctx.enter_context(tc.tile_pool(name="psum", bufs=6, space="PSUM"))
        psuml = ctx.enter_context(tc.tile_pool(name="psuml", bufs=2, space="PSUM"))

        # Pre-warm the PE during the startup DMA window.  HAM un-throttles
        # (1.2 -> 2.4 GHz) only after ~3.4us of SUSTAINED PE activity, so the
        # warmup burst must exceed that on its own; it also buys time for the
        # input DMA to build a few matmul-groups of slack, so the real stream
        # starts warm and never stutters (each stall costs a ~400ns pipeline
        # restart and risks re-throttling).  12 x N=512 cold matmuls ~ 4.5us.
        # The warmup tile is full wv-size and later ALIASED by wv_sb (same
        # tag, bufs=1): Tile's WAR dependency then defers wv's 2 MB DMA
        # until the last warmup matmul has read the buffer (~12us), keeping
        # it out of the startup-critical DMA window.
        warm_in = wpool.tile([P, KS, D], CDT, tag="warm", name="warm_in")
        if WARMUP_MMS:
            nc.vector.memset(warm_in[:, 0, 0:NH], 0.0)
            warm_ps = psum.tile([P, NH], F32, tag="mm", name="warm_ps")
            for _ in range(WARMUP_MMS):
                nc.tensor.matmul(
                    warm_ps[:],
                    lhsT=warm_in[:, 0, 0:128],
                    rhs=warm_in[:, 0, 0:NH],
                    start=True,
                    stop=True,
                )

        # Weights resident for the whole kernel; inputs for both batches
        # prefetched up front.  DMA issue order follows first use: the
        # first phase-A matmul group needs only m[:, :, 0:128] plus the
        # first i-half of batch-0 x (~1.25 MB), so those bytes are issued
        # first and the dense PE stream can start ~3.5us after DMA start.
        m_sb = wpool.tile([P, KS, D], CDT, tag="m", name="m_sb")
        wv_sb = wpool.tile([P, KS, D], CDT, tag="warm", name="wv_sb")  # aliases warm_in
        x_sbs = [
            iopool.tile([P, KS, C], CDT, tag="x", name="x_sb") for _ in range(BPC)
        ]
        c_sbs = [
            iopool.tile([P, KS, C], CDT, tag="c", name="c_sb") for _ in range(BPC)
        ]
        # Few LARGE dma_starts ordered by first use: each call costs ~0.65us
        # of HWDGE issue on the sync ring regardless of size (and a single
        # InstDMACopy is split across all 16 SDMA engines, so big calls
        # still run at full aggregate bandwidth).  Many small descriptors
        # are issue-rate-bound and starve the PE.
        nc.sync.dma_start(x_sbs[0][:, :, 0:NH], xT[0, :, :, 0:NH])
        nc.sync.dma_start(m_sb[:, :, 0:P], m[:, :, 0:P])  # phase-A group 0
        nc.sync.dma_start(m_sb[:, :, P : 3 * P], m[:, :, P : 3 * P])
        nc.sync.dma_start(m_sb[:, :, 3 * P : D], m[:, :, 3 * P : D])
        nc.sync.dma_start(x_sbs[0][:, :, NH:C], xT[0, :, :, NH:C])
        # (all DRAM tensors are partition-major [P, KS, cols] so src and
        #  dst APs enumerate elements in the same (p, ks, col) order)
        nc.sync.dma_start(c_sbs[0][:, :, :], cT[0])
        nc.sync.dma_start(wv_sb[:, :, :], wv[:])  # WAR-deferred past warmup
        for n in range(1, BPC):
            nc.sync.dma_start(x_sbs[n][:, :, :], xT[n])
            nc.sync.dma_start(c_sbs[n][:, :, :], cT[n])

        ones = wpool.tile([P, 1], CDT, tag="ones", name="ones")
        nc.vector.memset(ones[:], 1.0)

        for n in range(BPC):
            x_sb = x_sbs[n]
            c_sb = c_sbs[n]

            # ---- phase A: aT[o,i] = M.T @ xT ----
            # ih outer so the very first matmul group only needs the first
            # i-half of x + the first 128 columns of m (lands first).
            # N stays 512: narrower windows double the m-chunk consumption
            # rate and outrun the DMA (measured: gaps + HAM re-throttle).
            awins = [(0, NH), (NH, NH)]
            aT_sb = actpool.tile([P, KS, C], CDT, tag="aT", name="aT_sb")
            for istart, iw in awins:
                for ot in range(KS):
                    ps = psum.tile([P, NH], F32, tag="mm", name="ps_mm")
                    for ks in range(KS):
                        nc.tensor.matmul(
                            ps[:, 0:iw],
                            lhsT=m_sb[:, ks, ot * P : (ot + 1) * P],
                            rhs=x_sb[:, ks, istart : istart + iw],
                            start=(ks == 0),
                            stop=(ks == KS - 1),
                        )
                    nc.vector.tensor_copy(
                        aT_sb[:, ot, istart : istart + iw], ps[:, 0:iw]
                    )

            # ---- phase B: v[j,o] = cT.T @ WvT ----
            v_sb = actpool.tile([P, KS, D], CDT, tag="v", name="v_sb")
            for jt in range(NT):
                ps = [psum.tile([P, NH], F32, tag="mm", name="ps_mm") for _ in range(2)]
                for ks in range(KS):
                    for oh in range(2):
                        nc.tensor.matmul(
                            ps[oh][:],
                            lhsT=c_sb[:, ks, jt * P : (jt + 1) * P],
                            rhs=wv_sb[:, ks, oh * NH : (oh + 1) * NH],
                            start=(ks == 0),
                            stop=(ks == KS - 1),
                        )
                for oh in range(2):
                    nc.vector.tensor_copy(
                        v_sb[:, jt, oh * NH : (oh + 1) * NH], ps[oh][:]
                    )

            # ---- phase C: ST[j,i] = cT.T @ aT -> ET = exp(ST/32) ----
            eT_sb = actpool.tile([P, KS, C], CDT, tag="eT", name="eT_sb")
            for jt in range(NT):
                ps = [psum.tile([P, NH], F32, tag="mm", name="ps_mm") for _ in range(2)]
                for os_ in range(KS):
                    for ih in range(2):
                        nc.tensor.matmul(
                            ps[ih][:],
                            lhsT=c_sb[:, os_, jt * P : (jt + 1) * P],
                            rhs=aT_sb[:, os_, ih * NH : (ih + 1) * NH],
                            start=(os_ == 0),
                            stop=(os_ == KS - 1),
                        )
                for ih in range(2):
                    nc.scalar.activation(
                        eT_sb[:, jt, ih * NH : (ih + 1) * NH],
                        ps[ih][:],
                        mybir.ActivationFunctionType.Exp,
                        scale=SCALE,
                    )

            # ---- phase D: out'[i,o] = ET.T @ v ; l = ET.T @ ones ; scale ----
            # oh-major: the oh=0 half's scale+DMA overlaps the oh=1 matmuls,
            # shrinking the end-of-kernel tail.
            for it in range(NT):
                o_sb = outpool.tile([P, D], CDT, tag="o", name="o_sb")
                psl = psuml.tile([P, 1], F32, tag="l", name="ps_l")
                r_it = smpool.tile([P, 1], F32, tag="r", name="r_it")
                for oh in range(2):
                    ps = psum.tile([P, NH], F32, tag="mm", name="ps_mm")
                    for js in range(NT):
                        lhsT = eT_sb[:, js, it * P : (it + 1) * P]
                        nc.tensor.matmul(
                            ps[:],
                            lhsT=lhsT,
                            rhs=v_sb[:, js, oh * NH : (oh + 1) * NH],
                            start=(js == 0),
                            stop=(js == NT - 1),
                        )
                        if oh == 0:
                            nc.tensor.matmul(
                                psl[:],
                                lhsT=lhsT,
                                rhs=ones[:, 0:1],
                                start=(js == 0),
                                stop=(js == NT - 1),
                            )
                    if oh == 0:
                        nc.vector.reciprocal(r_it[:], psl[:])
                    nc.vector.tensor_scalar_mul(
                        o_sb[:, oh * NH : (oh + 1) * NH], ps[:], r_it[:]
                    )
                    nc.sync.dma_start(
                        out[n, it, :, oh * NH : (oh + 1) * NH],
                        o_sb[:, oh * NH : (oh + 1) * NH],
                    )


_NC_CACHE = {}


def _build():
    if "nc" in _NC_CACHE:
        return _NC_CACHE["nc"]
    nc = bacc.Bacc("TRN2", target_bir_lowering=False, debug=False)
    xT = nc.dram_tensor("xT", [BPC, P, KS, C], CDT, kind="ExternalInput").ap()
    cT = nc.dram_tensor("cT", [BPC, P, KS, C], CDT, kind="ExternalInput").ap()
    m = nc.dram_tensor("m", [P, KS, D], CDT, kind="ExternalInput").ap()
    wv = nc.dram_tensor("wv", [P, KS, D], CDT, kind="ExternalInput").ap()
    out = nc.dram_tensor("out", [BPC, NT, P, D], CDT, kind="ExternalOutput").ap()
    with tile.TileContext(nc) as tc:
        _emit(tc, xT, cT, m, wv, out)
    nc.compile()
    _NC_CACHE["nc"] = nc
    return nc


def kernel(**inputs) -> np.ndarray:
    x = np.asarray(inputs["x"], dtype=np.float32).reshape(B, C, D)
    cond = np.asarray(inputs["cond_img"], dtype=np.float32).reshape(B, C, D)
    Wq = np.asarray(inputs["Wq"], dtype=np.float32)
    Wkv = np.asarray(inputs["Wkv"], dtype=np.float32)

    # Host-side weight folding: scores = (x @ M) @ c.T with M = Wq.T @ Wk.
    M = (Wq.T.astype(np.float64) @ Wkv[:D].astype(np.float64)).astype(np.float32)

    # Pre-transpose on host so the contraction dim lands on partitions.
    xT = np.ascontiguousarray(x.transpose(0, 2, 1)).astype(NPDT)  # (B, D, C)
    cT = np.ascontiguousarray(cond.transpose(0, 2, 1)).astype(NPDT)
    wvT = np.ascontiguousarray(Wkv[D:].T).astype(NPDT)  # (D_in, D_out)

    # Partition-major device layouts: [..., P, KS, cols] with d = ks*P + p,
    # so one big DMA call enumerates (p, ks, col) identically on both sides.
    xT = np.ascontiguousarray(
        xT.reshape(NCORES, BPC, KS, P, C).transpose(0, 1, 3, 2, 4)
    )
    cT = np.ascontiguousarray(
        cT.reshape(NCORES, BPC, KS, P, C).transpose(0, 1, 3, 2, 4)
    )
    m = np.ascontiguousarray(M.astype(NPDT).reshape(KS, P, D).transpose(1, 0, 2))
    wv = np.ascontiguousarray(wvT.reshape(KS, P, D).transpose(1, 0, 2))

    in_maps = [
        {"xT": xT[i], "cT": cT[i], "m": m, "wv": wv}
        for i in range(NCORES)
    ]

    nc = _build()
    trace = bool(os.environ.get("KERNEL_TRACE"))
    res = bass_utils.run_bass_kernel_spmd(
        nc, in_maps, core_ids=list(range(NCORES)), trace=trace
    )
    if trace:
        _NC_CACHE["last_result"] = res

    outs = np.stack([np.asarray(res.results[i]["out"]) for i in range(NCORES)])
    return outs.reshape(B, C, HH, WW).astype(np.float32)
